# revision 1
# baseline (speedup 1.0000x reference)
"""Trainium2 Bass kernel for nn_NodeFeat_77841987272986 (2-hop GNN message passing).

Computation (reference):
    x3 = [x, x*rsqrt(deg), x*sqrt(deg)]                  # [N, 3D]
    y1 = (1/deg) * segsum(x3[col], row)                  # [N, 3D]
    y2 = (1/deg) * segsum(y1[col], row) - x3             # [N, 3D]
    out = concat([x3, y1, y2], 1).reshape(N, 9, D).transpose(0, 2, 1)  # [N, D, 9]

Strategy: 1-D node-parallel over 8 cores (12500 dest nodes each).  Edges are
sharded by destination, sorted by (dest-tile, col-bucket, col).  Each SpMM is
computed as a sequence of one-hot matmuls on the TensorEngine: for each chunk
of 128 edges, gather source rows with dma_gather (int16 indices, 32768-row
col buckets), build a one-hot [128 edges x 128 dest-slots] selection matrix
with a single is_equal op against an iota constant, and accumulate
psum[dest, :] += onehot.T @ gathered on the PE.  y1 shards are AllGathered
between hops.  All padding edges carry rowmod=-1 so their one-hot column is
zero and they contribute nothing.
"""
import sys
sys.path.insert(0, '/opt/trn_rl_repo')
import numpy as np

P = 128            # partitions / dest-tile size / edge-chunk size
BUCKET = 32768     # int16-addressable gather window (rows)
GROUP = 2          # dest tiles per gather group

# full-problem constants (hardcoded per the task contract)
FULL_N = 100000
FULL_D = 64
FULL_CORES = 8


# ----------------------------------------------------------------- host prep

def _plan(N, D, n_cores, row, col, deg, x):
    """Shared static structure + per-core input arrays."""
    npc = N // n_cores
    n_tiles = (npc + P - 1) // P
    n_groups = (n_tiles + GROUP - 1) // GROUP
    n_buckets = (N + BUCKET - 1) // BUCKET

    deg = deg.reshape(-1).astype(np.float64)
    sr = (1.0 / np.sqrt(deg)).astype(np.float32)
    sq = np.sqrt(deg).astype(np.float32)
    dr = (1.0 / deg).astype(np.float32)
    x3f = np.concatenate([x, x * sr[:, None], x * sq[:, None]], axis=1)
    x3f = np.ascontiguousarray(x3f, dtype=np.float32)

    # per-core sorted edge lists
    core_of = row // npc
    per_core = []
    cnts = np.zeros((n_cores, n_tiles, n_buckets), dtype=np.int64)
    for c in range(n_cores):
        m = core_of == c
        er = row[m] - c * npc
        ec = col[m]
        t = er // P
        b = ec // BUCKET
        order = np.lexsort((ec, b, t))
        er, ec, t, b = er[order], ec[order], t[order], b[order]
        np.add.at(cnts[c], (t, b), 1)
        per_core.append((er, ec, t, b))

    # shared static chunk structure: K[t][b] = max over cores of ceil(cnt/128)
    K = np.ceil(cnts.max(axis=0) / P).astype(np.int64)          # [n_tiles, nb]
    for t in range(n_tiles):
        if K[t].sum() == 0:
            K[t][0] = 1      # ensure every tile has >=1 chunk (psum gets written)

    # group structure: for g, for b, for t in tiles(g): K[t][b] chunks
    groups = []      # list of (g, [(b, [(t, Ktb, pos0), ...], gb_pos0, gb_K)])
    pos = 0
    for g in range(n_groups):
        tiles_g = list(range(g * GROUP, min((g + 1) * GROUP, n_tiles)))
        bl = []
        for b in range(n_buckets):
            ent = []
            gb_pos0 = pos
            for t in tiles_g:
                k = int(K[t][b])
                if k:
                    ent.append((t, k, pos))
                    pos += k
            bl.append((b, ent, gb_pos0, pos - gb_pos0))
        groups.append((g, tiles_g, bl))
    CHT = pos            # total chunks (global, same for each core)

    # per-core packed arrays in the global chunk order
    ins = []
    for c in range(n_cores):
        er, ec, t, b = per_core[c]
        idx = np.zeros(CHT * P, dtype=np.int16)
        rowmod = np.full(CHT * P, -1.0, dtype=np.float32)
        # segment starts per (t, b) in the sorted arrays
        seg_start = {}
        starts = np.searchsorted(t * n_buckets + b,
                                 np.arange(n_tiles * n_buckets))
        counts = np.diff(np.append(starts, len(t)))
        for g, tiles_g, bl in groups:
            for bb, ent, gb_pos0, gb_K in bl:
                for (tt, k, pos0) in ent:
                    s = starts[tt * n_buckets + bb]
                    n = counts[tt * n_buckets + bb]
                    assert n <= k * P
                    sl = slice(pos0 * P, pos0 * P + n)
                    idx[sl] = (ec[s:s + n] - bb * BUCKET).astype(np.int16)
                    rowmod[sl] = (er[s:s + n] - tt * P).astype(np.float32)
        # wrap idx into dma_gather layout: [128, CHT*8]; i -> [i%16, i//16], x8
        iw = idx.reshape(CHT, 8, 16).transpose(2, 0, 1)          # [16, CHT, 8]
        iw = np.tile(iw.reshape(16, CHT * 8), (8, 1))            # [128, CHT*8]
        # rowmod layout [128, CHT]: chunk C slot p = edge C*128+p
        rm = rowmod.reshape(CHT, P).T.copy()
        # per-dest arrays
        base = c * npc
        drw = np.ones((P, n_tiles), dtype=np.float32)
        dd = dr[base:base + npc]
        drw.T.reshape(-1)[:npc] = dd            # [t, p] order -> row t*128+p
        drw = drw  # drw[p, t] after transpose below
        drw = np.ones((P, n_tiles), dtype=np.float32)
        tmp = np.ones(n_tiles * P, dtype=np.float32)
        tmp[:npc] = dd
        drw = tmp.reshape(n_tiles, P).T.copy()
        x3o = np.ascontiguousarray(x3f[base:base + npc])
        ins.append({
            "idx16": np.ascontiguousarray(iw),
            "rowmod": np.ascontiguousarray(rm),
            "drw": drw,
            "x3o": x3o,
            "x3f": x3f,
        })
    static = dict(N=N, D=D, n_cores=n_cores, npc=npc, n_tiles=n_tiles,
                  n_buckets=n_buckets, groups=groups, K=K, CHT=CHT)
    return static, ins


# ------------------------------------------------------------- device kernel

def _build(static, reps=1, variant='full'):
    import concourse.bass as bass
    import concourse.bacc as bacc
    import concourse.mybir as mybir
    import concourse.tile as tile

    N = static["N"]; D = static["D"]; n_cores = static["n_cores"]
    npc = static["npc"]; n_tiles = static["n_tiles"]
    groups = static["groups"]; CHT = static["CHT"]
    D3 = 3 * D
    f32 = mybir.dt.float32

    Kg_max = max(sum(k for _, ent, _, gbk in bl for (_, k, _) in ent)
                 for _, _, bl in groups)
    Kgb_max = max(gbk for _, _, bl in groups for (_, ent, _, gbk) in bl)

    nc = bacc.Bacc("TRN2", target_bir_lowering=False, debug=False,
                   num_devices=n_cores)
    x3f = nc.dram_tensor("x3f", [N, D3], f32, kind="ExternalInput")
    x3o_d = nc.dram_tensor("x3o", [npc, D3], f32, kind="ExternalInput")
    idx16_d = nc.dram_tensor("idx16", [P, CHT * 8], mybir.dt.int16,
                             kind="ExternalInput")
    rowmod_d = nc.dram_tensor("rowmod", [P, CHT], f32, kind="ExternalInput")
    drw_d = nc.dram_tensor("drw", [P, n_tiles], f32, kind="ExternalInput")
    out_d = nc.dram_tensor("out", [npc, 9 * D], f32, kind="ExternalOutput")
    y1sh = nc.dram_tensor("y1sh", [npc, D3], f32)
    y1f = nc.dram_tensor("y1f", [N, D3], f32,
                         addr_space="Shared" if n_cores > 1 else "Local")

    def rows_of(t):
        return min(P, npc - t * P)

    with tile.TileContext(nc) as tc:
        with tc.tile_pool(name="res", bufs=1) as res, \
             tc.tile_pool(name="gat", bufs=2) as gat, \
             tc.tile_pool(name="oh", bufs=3) as ohp, \
             tc.tile_pool(name="ep", bufs=3) as ep, \
             tc.tile_pool(name="ps", bufs=4, space="PSUM") as psp:

            idx16 = res.tile([P, CHT * 8], mybir.dt.int16)
            nc.sync.dma_start(out=idx16[:], in_=idx16_d[:])
            rowmod = res.tile([P, CHT], f32)
            nc.sync.dma_start(out=rowmod[:], in_=rowmod_d[:])
            drw = res.tile([P, n_tiles], f32)
            nc.sync.dma_start(out=drw[:], in_=drw_d[:])
            iota_i = res.tile([P, P], mybir.dt.int32)
            nc.gpsimd.iota(iota_i[:], pattern=[[1, P]], base=0,
                           channel_multiplier=0)
            iota_f = res.tile([P, P], f32)
            nc.vector.tensor_copy(iota_f[:], iota_i[:])

            def hop(table_ap, store):
                """store(t, psum_ap) writes out tile t's epilogue."""
                for g, tiles_g, bl in groups:
                    Kg = sum(k for _, ent, _, _ in bl for (_, k, _) in ent)
                    base_pos = min(p0 for _, ent, p0, _ in bl)
                    rhs = gat.tile([P, Kg_max, D3], f32, tag="rhs")
                    # gathers per (g, b)
                    for b, ent, gb_pos0, gb_K in bl:
                        if gb_K == 0:
                            continue
                        lo = b * BUCKET
                        hi = min(N, lo + BUCKET)
                        ni = gb_K * P
                        nc.gpsimd.dma_gather(
                            out_ap=rhs[:, gb_pos0 - base_pos:
                                       gb_pos0 - base_pos + gb_K, :],
                            in_ap=table_ap[lo:hi, :],
                            idxs_ap=idx16[:, (gb_pos0 * 8):(gb_pos0 * 8 + ni // 16)],
                            num_idxs=ni, num_idxs_reg=ni,
                            elem_size=D3, single_packet=False)
                    # psums for the tiles of this group
                    psums = {}
                    for t in tiles_g:
                        pst = psp.tile([P, D3], f32, tag="ps", name=f"ps{t}")
                        psums[t] = pst
                    first = {t: True for t in tiles_g}
                    last_pos = {}
                    for b, ent, gb_pos0, gb_K in bl:
                        for (t, k, pos0) in ent:
                            last_pos[t] = pos0 + k - 1
                    # one-hot + matmuls per (g, b)
                    for b, ent, gb_pos0, gb_K in bl:
                        if gb_K == 0 or variant == 'gathers':
                            continue
                        oht = ohp.tile([P, Kgb_max, P], f32, tag="oh")
                        rm_sl = rowmod[:, gb_pos0:gb_pos0 + gb_K]
                        in0 = bass.AP(iota_f.tensor, iota_f[:].offset,
                                      [iota_f[:].ap[0], [0, gb_K],
                                       iota_f[:].ap[-1]])
                        in1 = bass.AP(rowmod.tensor, rm_sl.offset,
                                      [rm_sl.ap[0], [rm_sl.ap[-1][0], gb_K],
                                       [0, P]])
                        nc.vector.tensor_tensor(
                            out=oht[:, :gb_K, :], in0=in0, in1=in1,
                            op=mybir.AluOpType.is_equal)
                        for (t, k, pos0) in ent:
                            for j in range(k):
                                pos = pos0 + j
                                nc.tensor.matmul(
                                    out=psums[t][:],
                                    lhsT=oht[:, pos - gb_pos0, :],
                                    rhs=rhs[:, pos - base_pos, :],
                                    start=first[t],
                                    stop=(pos == last_pos[t]))
                                first[t] = False
                    for t in tiles_g:
                        if variant != 'gathers':
                            store(t, psums[t])

            # ---- hop 1
            def store1(t, ps):
                r = rows_of(t)
                y1d = ep.tile([P, D3], f32, tag="y1d", name="y1d")
                nc.vector.tensor_scalar(
                    out=y1d[:], in0=ps[:], scalar1=drw[:, t:t + 1],
                    scalar2=None, op0=mybir.AluOpType.mult)
                nc.scalar.dma_start(out=y1sh[t * P: t * P + r, :],
                                    in_=y1d[:r, :])
            def run_pipeline():
                hop(x3f[:], store1)
                # ---- allgather y1
                if n_cores > 1:
                    nc.gpsimd.collective_compute(
                        "AllGather", mybir.AluOpType.bypass,
                        replica_groups=[list(range(n_cores))],
                        ins=[y1sh[:]], outs=[y1f[:]])
                else:
                    nc.sync.dma_start(out=y1f[:], in_=y1sh[:])

            # ---- hop 2
            def store2(t, ps):
                r = rows_of(t)
                x3t = ep.tile([P, D3], f32, tag="x3t")
                nc.sync.dma_start(out=x3t[:r, :],
                                  in_=x3o_d[t * P: t * P + r, :])
                y1t = ep.tile([P, D3], f32, tag="y1t")
                nc.sync.dma_start(out=y1t[:r, :],
                                  in_=y1sh[t * P: t * P + r, :])
                y2d = ep.tile([P, D3], f32, tag="y2d")
                nc.vector.tensor_scalar(
                    out=y2d[:], in0=ps[:], scalar1=drw[:, t:t + 1],
                    scalar2=None, op0=mybir.AluOpType.mult)
                nc.vector.tensor_tensor(out=y2d[:r, :], in0=y2d[:r, :],
                                        in1=x3t[:r, :],
                                        op=mybir.AluOpType.subtract)
                ot = ep.tile([P, D, 9], f32, tag="ot")
                nc.vector.tensor_copy(
                    out=ot[:r, :, 0:3],
                    in_=x3t[:r, :].rearrange("p (v f) -> p f v", v=3))
                nc.vector.tensor_copy(
                    out=ot[:r, :, 3:6],
                    in_=y1t[:r, :].rearrange("p (v f) -> p f v", v=3))
                nc.vector.tensor_copy(
                    out=ot[:r, :, 6:9],
                    in_=y2d[:r, :].rearrange("p (v f) -> p f v", v=3))
                nc.scalar.dma_start(
                    out=out_d[t * P: t * P + r, :],
                    in_=ot[:r, :, :].rearrange("p f v -> p (f v)"))

            def store_g(t, ps):
                if t == 0:
                    z = ep.tile([P, D3], f32, tag="y1d", name="zz")
                    nc.vector.tensor_copy(z[:], ps[:])
                    nc.sync.dma_start(out=out_d[0:P, 0:D3], in_=z[:])

            for _rep in range(reps):
                if variant == 'gathers':
                    hop(x3f[:], store_g)
                    hop(x3f[:], store_g)
                    continue
                run_pipeline()
                if variant != 'hop1':
                    hop(y1f[:], store2)

    nc.compile()
    return nc


# ----------------------------------------------------------------- interface

_CACHE = {}

def _get_nc(static):
    key = (static["N"], static["D"], static["n_cores"], static["CHT"],
           tuple(static["K"].reshape(-1).tolist()))
    if key not in _CACHE:
        _CACHE[key] = _build(static)
    return _CACHE[key]


def kernel(x, deg, row, col):
    from concourse.bass_utils import run_bass_kernel_spmd
    x = np.asarray(x, dtype=np.float32)
    deg = np.asarray(deg, dtype=np.float32)
    row = np.asarray(row).astype(np.int64)
    col = np.asarray(col).astype(np.int64)
    N, D = x.shape
    n_cores = FULL_CORES
    static, ins = _plan(N, D, n_cores, row, col, deg, x)
    nc = _get_nc(static)
    in_maps = [{"x3f": m["x3f"], "x3o": m["x3o"], "idx16": m["idx16"],
                "rowmod": m["rowmod"], "drw": m["drw"]} for m in ins]
    try:
        res = run_bass_kernel_spmd(nc, in_maps, core_ids=list(range(n_cores)))
    except Exception:
        res = run_bass_kernel_spmd(nc, in_maps, core_ids=list(range(n_cores)))
    out = np.concatenate([res.results[c]["out"] for c in range(n_cores)],
                         axis=0)
    return out.reshape(N, D, 9).astype(np.float32)



# revision 5
# speedup vs baseline: 10.1105x; 10.1105x over previous
"""Trainium2 Bass kernel for nn_NodeFeat_77841987272986 (2-hop GNN message passing).

Computation (reference):
    x3 = [x, x*rsqrt(deg), x*sqrt(deg)]                  # [N, 3D]
    y1 = (1/deg) * segsum(x3[col], row)                  # [N, 3D]
    y2 = (1/deg) * segsum(y1[col], row) - x3             # [N, 3D]
    out = concat([x3, y1, y2], 1).reshape(N, 9, D).transpose(0, 2, 1)  # [N, D, 9]

v2 design (vs the fp32 baseline):
  - Gather tables (x3 for hop 1, y1 for hop 2) are bf16 padded to 256
    elements = 512 B rows: full-rate DMA descriptors (no <512B penalty)
    at half the fp32-768B byte count.
  - One-hot matrices and matmuls in bf16: 1 PE cycle/row instead of 4.
  - Device emits only the y1 shard (bf16, doubles as the AllGather input)
    and the y2 shard; the host assembles the final [N, D, 9] f32 output
    (the x3 third comes from host-exact fp32).
  - The AllGather can be split into n_ag chunks issued between hop-1
    groups so the collective overlaps hop-1 compute.  Table rows are
    permuted (chunk-major, then core-major) so one idx table serves both
    hops.
"""
import sys
sys.path.insert(0, '/opt/trn_rl_repo')
import numpy as np
import ml_dtypes

BF16 = ml_dtypes.bfloat16

P = 128            # partitions / dest-tile size / edge-chunk size
BUCKET = 32768     # int16-addressable gather window (rows)
GROUP = 4          # dest tiles per gather group
N_AG = 1           # AllGather chunks (1 = single collective after hop 1)
DPAD = 256         # padded feature row (bf16 elements) = 512 B
N_QUEUES = 4       # SWDGE queues for dma_gather round-robin

FULL_N = 100000
FULL_D = 64
FULL_CORES = 8


# ----------------------------------------------------------------- host prep

def _ag_layout(npc, n_tiles, n_ag, n_groups):
    """Split tiles into n_ag spans aligned to GROUP boundaries.
    Returns list of (t0, t1, row0, rows, off) with off = global row offset
    of the span in the gathered table (8 cores concatenated per span)."""
    spans = []
    gper = (n_groups + n_ag - 1) // n_ag
    off = 0
    for k in range(n_ag):
        g0, g1 = k * gper, min((k + 1) * gper, n_groups)
        if g0 >= g1:
            continue
        t0, t1 = g0 * GROUP, min(g1 * GROUP, n_tiles)
        r0 = t0 * P
        r1 = min(t1 * P, npc)
        spans.append((t0, t1, r0, r1 - r0, off))
        off += FULL_CORES * (r1 - r0)
    return spans


def _plan(N, D, n_cores, row, col, deg, x, n_ag=N_AG):
    npc = N // n_cores
    n_tiles = (npc + P - 1) // P
    n_groups = (n_tiles + GROUP - 1) // GROUP
    n_buckets = (N + BUCKET - 1) // BUCKET

    deg = deg.reshape(-1).astype(np.float64)
    sr = (1.0 / np.sqrt(deg)).astype(np.float32)
    sq = np.sqrt(deg).astype(np.float32)
    dr = (1.0 / deg).astype(np.float32)
    x = x.astype(np.float32)
    x3f = np.concatenate([x, x * sr[:, None], x * sq[:, None]], axis=1)
    x3f = np.ascontiguousarray(x3f, dtype=np.float32)      # host-exact [N, 3D]

    spans = _ag_layout(npc, n_tiles, n_ag, n_groups)
    # permuted position of node (c, i):  span k containing i, then
    # off_k + c*rows_k + (i - row0_k)
    perm = np.empty(N, dtype=np.int64)
    ii = np.arange(npc)
    pos_in_shard = np.empty(npc, dtype=np.int64)
    shard_off = np.empty(npc, dtype=np.int64)
    shard_rows = np.empty(npc, dtype=np.int64)
    for (t0, t1, r0, rows, off) in spans:
        sl = slice(r0, r0 + rows)
        pos_in_shard[sl] = ii[sl] - r0
        shard_off[sl] = off
        shard_rows[sl] = rows
    for c in range(n_cores):
        perm[c * npc: (c + 1) * npc] = shard_off + c * shard_rows + pos_in_shard

    # padded bf16 gather table for hop 1, rows in permuted order
    x3pad = np.zeros((N, DPAD), dtype=BF16)
    x3pad[perm, :3 * D] = x3f.astype(BF16)

    # per-core sorted edge lists (dest-sharded); cols mapped through perm
    pcol = perm[col]
    core_of = row // npc
    per_core = []
    cnts = np.zeros((n_cores, n_tiles, n_buckets), dtype=np.int64)
    for c in range(n_cores):
        m = core_of == c
        er = row[m] - c * npc
        ec = pcol[m]
        t = er // P
        b = ec // BUCKET
        order = np.lexsort((ec, b, t))
        er, ec, t, b = er[order], ec[order], t[order], b[order]
        np.add.at(cnts[c], (t, b), 1)
        per_core.append((er, ec, t, b))

    K = np.ceil(cnts.max(axis=0) / P).astype(np.int64)          # [n_tiles, nb]
    for t in range(n_tiles):
        if K[t].sum() == 0:
            K[t][0] = 1

    groups = []      # (g, tiles_g, [(b, [(t, Ktb, pos0), ...], gb_pos0, gb_K)])
    pos = 0
    for g in range(n_groups):
        tiles_g = list(range(g * GROUP, min((g + 1) * GROUP, n_tiles)))
        bl = []
        for b in range(n_buckets):
            ent = []
            gb_pos0 = pos
            for t in tiles_g:
                k = int(K[t][b])
                if k:
                    ent.append((t, k, pos))
                    pos += k
            bl.append((b, ent, gb_pos0, pos - gb_pos0))
        groups.append((g, tiles_g, bl))
    CHT = pos

    # entry order shared with _build's gather loop: (g, b, t) nesting
    NE = sum(len(ent) for _, _, bl in groups for _, ent, _, _ in bl)

    ins = []
    for c in range(n_cores):
        er, ec, t, b = per_core[c]
        # pad slots are -1: the DGE generates descriptors only for the
        # leading `gbc` real indices of each per-(t,b) gather (15.8% fewer
        # descriptors than fetching pad rows)
        idx = np.full(CHT * P, -1, dtype=np.int16)
        rowmod = np.full(CHT * P, -1.0, dtype=np.float32)
        gbc = np.zeros(NE, dtype=np.int32)
        starts = np.searchsorted(t * n_buckets + b,
                                 np.arange(n_tiles * n_buckets))
        counts = np.diff(np.append(starts, len(t)))
        ei = 0
        for g, tiles_g, bl in groups:
            for bb, ent, gb_pos0, gb_K in bl:
                for (tt, k, pos0) in ent:
                    s = starts[tt * n_buckets + bb]
                    n = counts[tt * n_buckets + bb]
                    assert n <= k * P
                    sl = slice(pos0 * P, pos0 * P + n)
                    idx[sl] = (ec[s:s + n] - bb * BUCKET).astype(np.int16)
                    rowmod[sl] = (er[s:s + n] - tt * P).astype(np.float32)
                    if n == 0:
                        # the DGE needs >=1 valid leading index
                        idx[pos0 * P] = 0
                        n = 1
                    gbc[ei] = n
                    ei += 1
        assert ei == NE
        iw = idx.reshape(CHT, 8, 16).transpose(2, 0, 1)          # [16, CHT, 8]
        iw = np.tile(iw.reshape(16, CHT * 8), (8, 1))            # [128, CHT*8]
        rm = rowmod.reshape(CHT, P).T.copy()
        base = c * npc
        dd = dr[base:base + npc]
        tmp = np.ones(n_tiles * P, dtype=np.float32)
        tmp[:npc] = dd
        drw = tmp.reshape(n_tiles, P).T.copy()
        x3o = np.ascontiguousarray(x3f[base:base + npc]).astype(BF16)
        ins.append({
            "idx16": np.ascontiguousarray(iw),
            "rowmod": np.ascontiguousarray(rm),
            "drw": drw,
            "x3o": x3o,
            "x3pad": x3pad,
            "gbc": gbc.reshape(1, NE),
        })
    static = dict(N=N, D=D, n_cores=n_cores, npc=npc, n_tiles=n_tiles,
                  n_buckets=n_buckets, groups=groups, K=K, CHT=CHT,
                  spans=spans, n_ag=n_ag, NE=NE)
    return static, ins


# ------------------------------------------------------------- device kernel

def _build(static, reps=1, variant='full'):
    import concourse.bass as bass
    import concourse.bacc as bacc
    import concourse.mybir as mybir
    import concourse.tile as tile

    N = static["N"]; D = static["D"]; n_cores = static["n_cores"]
    npc = static["npc"]; n_tiles = static["n_tiles"]
    groups = static["groups"]; CHT = static["CHT"]
    spans = static["spans"]
    D3 = 3 * D
    f32 = mybir.dt.float32
    bf16 = mybir.dt.bfloat16

    Kg_max = max(sum(k for _, ent, _, gbk in bl for (_, k, _) in ent)
                 for _, _, bl in groups)
    Kgb_max = max(gbk for _, _, bl in groups for (_, ent, _, gbk) in bl)

    n_queues = {'g2q': 2, 'full2q': 2, 'g4q': 4, 'noag4q': 4, 'hop14q': 4,
                'g1q': 1, 'noag1q': 1}.get(variant, N_QUEUES)
    nc = bacc.Bacc("TRN2", target_bir_lowering=False, debug=False,
                   num_devices=n_cores, num_swdge_queues=n_queues)
    x3pad_d = nc.dram_tensor("x3pad", [N, DPAD], bf16, kind="ExternalInput")
    x3o_d = nc.dram_tensor("x3o", [npc, D3], bf16, kind="ExternalInput")
    idx16_d = nc.dram_tensor("idx16", [P, CHT * 8], mybir.dt.int16,
                             kind="ExternalInput")
    rowmod_d = nc.dram_tensor("rowmod", [P, CHT], f32, kind="ExternalInput")
    drw_d = nc.dram_tensor("drw", [P, n_tiles], f32, kind="ExternalInput")
    NE = static["NE"]
    gbc_d = nc.dram_tensor("gbc", [1, NE], mybir.dt.int32,
                           kind="ExternalInput")
    y1sh = nc.dram_tensor("y1sh", [npc, DPAD], bf16)
    y1o = nc.dram_tensor("y1o", [npc, DPAD], bf16, kind="ExternalOutput")
    y2o = nc.dram_tensor("y2o", [npc, D3], bf16, kind="ExternalOutput")
    y1f = nc.dram_tensor("y1f", [N, DPAD], bf16,
                         addr_space="Shared" if n_cores > 1 else "Local")

    # tile span -> ag span index (for chunked collective issue)
    last_group_of_span = {}
    for si, (t0, t1, r0, rows, off) in enumerate(spans):
        last_group_of_span[(t1 + GROUP - 1) // GROUP - 1] = si

    def rows_of(t):
        return min(P, npc - t * P)

    with tile.TileContext(nc) as tc:
        with tc.tile_pool(name="res", bufs=1) as res, \
             tc.tile_pool(name="gat", bufs=2) as gat, \
             tc.tile_pool(name="oh", bufs=3) as ohp, \
             tc.tile_pool(name="ep", bufs=3) as ep, \
             tc.tile_pool(name="ps", bufs=8, space="PSUM") as psp:

            idx16 = res.tile([P, CHT * 8], mybir.dt.int16)
            nc.sync.dma_start(out=idx16[:], in_=idx16_d[:])
            rowmod = res.tile([P, CHT], f32)
            nc.sync.dma_start(out=rowmod[:], in_=rowmod_d[:])
            drw = res.tile([P, n_tiles], f32)
            nc.sync.dma_start(out=drw[:], in_=drw_d[:])
            iota_i = res.tile([P, P], mybir.dt.int32)
            nc.gpsimd.iota(iota_i[:], pattern=[[1, P]], base=0,
                           channel_multiplier=0)
            iota_f = res.tile([P, P], f32)
            nc.vector.tensor_copy(iota_f[:], iota_i[:])
            cnt_t = res.tile([1, NE], mybir.dt.int32)
            nc.sync.dma_start(out=cnt_t[:], in_=gbc_d[:])
            creg = nc.gpsimd.alloc_register("gcnt")

            gather_only = variant in ('gathers', 'gzero', 'gxonly', 'g2q',
                                      'g4q')
            idx_use = idx16
            if variant == 'gzero':
                idx0 = res.tile([P, CHT * 8], mybir.dt.int16)
                nc.vector.memset(idx0[:], 0)
                idx_use = idx0
            gelem = 128 if variant == 'gxonly' else DPAD
            gq = [0]

            # slots past each gather's real count are never written by the
            # DGE (skipped -1 pads); zero both rhs ring buffers once so the
            # stale data the PE sees (x * onehot 0-rows) is always finite
            for _w in range(2):
                rw = gat.tile([P, Kg_max, gelem], bf16, tag="rhs")
                nc.vector.memset(rw[:], 0.0)

            def hop(table_ap, store, after_group=None):
                ei = [0]

                def entry_gathers(bl, rhs, base_pos):
                    # one gather per (tile, bucket) entry: trailing -1 pad
                    # indices are skipped by the DGE when the per-core real
                    # count is passed via register
                    for b, ent, gb_pos0, gb_K in bl:
                        lo = b * BUCKET
                        hi = min(N, lo + BUCKET)
                        for (t, k, pos0) in ent:
                            ni = k * P
                            e = ei[0]
                            ei[0] += 1
                            if variant == 'gzero':
                                cnt = ni
                            else:
                                nc.gpsimd.reg_load(creg, cnt_t[0:1, e:e + 1])
                                cnt = creg
                            gq[0] = (gq[0] + 1) % n_queues
                            nc.gpsimd.dma_gather(
                                out_ap=rhs[:, pos0 - base_pos:
                                           pos0 - base_pos + k, :],
                                in_ap=(table_ap[lo:hi, :] if gelem == DPAD
                                       else table_ap[lo:hi, 0:gelem]),
                                idxs_ap=idx_use[:, (pos0 * 8):
                                                (pos0 * 8 + ni // 16)],
                                num_idxs=ni, num_idxs_reg=cnt,
                                elem_size=gelem,
                                elem_step=(None if gelem == DPAD else DPAD),
                                single_packet=False, queue_num=gq[0])

                for g, tiles_g, bl in groups:
                    Kg = sum(k for _, ent, _, _ in bl for (_, k, _) in ent)
                    base_pos = min(p0 for _, ent, p0, _ in bl)
                    rhs = gat.tile([P, Kg_max, gelem], bf16, tag="rhs")
                    entry_gathers(bl, rhs, base_pos)
                    psums = {}
                    for t in tiles_g:
                        psums[t] = psp.tile([P, D3], f32, tag="ps",
                                            name=f"ps{t}")
                    first = {t: True for t in tiles_g}
                    last_pos = {}
                    for b, ent, gb_pos0, gb_K in bl:
                        for (t, k, pos0) in ent:
                            last_pos[t] = pos0 + k - 1
                    for b, ent, gb_pos0, gb_K in bl:
                        if gb_K == 0 or gather_only:
                            continue
                        oht = ohp.tile([P, Kgb_max, P], bf16, tag="oh")
                        rm_sl = rowmod[:, gb_pos0:gb_pos0 + gb_K]
                        in0 = bass.AP(iota_f.tensor, iota_f[:].offset,
                                      [iota_f[:].ap[0], [0, gb_K],
                                       iota_f[:].ap[-1]])
                        in1 = bass.AP(rowmod.tensor, rm_sl.offset,
                                      [rm_sl.ap[0], [rm_sl.ap[-1][0], gb_K],
                                       [0, P]])
                        nc.vector.tensor_tensor(
                            out=oht[:, :gb_K, :], in0=in0, in1=in1,
                            op=mybir.AluOpType.is_equal)
                        for (t, k, pos0) in ent:
                            for j in range(k):
                                pos = pos0 + j
                                nc.tensor.matmul(
                                    out=psums[t][:],
                                    lhsT=oht[:, pos - gb_pos0, :],
                                    rhs=rhs[:, pos - base_pos, 0:D3],
                                    start=first[t],
                                    stop=(pos == last_pos[t]))
                                first[t] = False
                    for t in tiles_g:
                        if not gather_only:
                            store(t, psums[t])
                    if after_group is not None:
                        after_group(g)

            # ---- hop 1
            def store1(t, ps):
                r = rows_of(t)
                y1d = ep.tile([P, DPAD], bf16, tag="y1d", name="y1d")
                nc.vector.tensor_scalar(
                    out=y1d[:, 0:D3], in0=ps[:], scalar1=drw[:, t:t + 1],
                    scalar2=None, op0=mybir.AluOpType.mult)
                nc.vector.memset(y1d[:, D3:DPAD], 0.0)
                nc.scalar.dma_start(out=y1sh[t * P: t * P + r, :],
                                    in_=y1d[:r, :])
                nc.scalar.dma_start(out=y1o[t * P: t * P + r, :],
                                    in_=y1d[:r, :])

            def ag_span(si):
                t0, t1, r0, rows, off = spans[si]
                if n_cores > 1:
                    nc.gpsimd.collective_compute(
                        "AllGather", mybir.AluOpType.bypass,
                        replica_groups=[list(range(n_cores))],
                        ins=[y1sh[r0:r0 + rows, :]],
                        outs=[y1f[off:off + n_cores * rows, :]])
                else:
                    nc.sync.dma_start(out=y1f[off:off + rows, :],
                                      in_=y1sh[r0:r0 + rows, :])

            def after_group1(g):
                if variant != 'full' and variant != 'full2q':
                    return
                si = last_group_of_span.get(g)
                if si is not None:
                    ag_span(si)

            # ---- hop 2
            def store2(t, ps):
                r = rows_of(t)
                x3t = ep.tile([P, D3], bf16, tag="x3t")
                nc.sync.dma_start(out=x3t[:r, :],
                                  in_=x3o_d[t * P: t * P + r, :])
                y2d = ep.tile([P, D3], bf16, tag="y2d")
                nc.vector.tensor_scalar(
                    out=y2d[:], in0=ps[:], scalar1=drw[:, t:t + 1],
                    scalar2=None, op0=mybir.AluOpType.mult)
                nc.vector.tensor_tensor(out=y2d[:r, :], in0=y2d[:r, :],
                                        in1=x3t[:r, :],
                                        op=mybir.AluOpType.subtract)
                nc.scalar.dma_start(out=y2o[t * P: t * P + r, :],
                                    in_=y2d[:r, :])

            def body():
                if variant == 'empty':
                    return
                if gather_only:
                    hop(x3pad_d[:], store1)
                    hop(x3pad_d[:], store1)
                    return
                hop(x3pad_d[:], store1, after_group=after_group1)
                if variant in ('hop1', 'hop14q'):
                    return
                if variant in ('noag', 'noag4q'):
                    # hop2 from x3pad (skips the collective dependency);
                    # numerically wrong, timing-only
                    hop(x3pad_d[:], store2)
                    return
                hop(y1f[:], store2)

            if reps == 1:
                body()
            else:
                with tc.For_i(0, reps) as _i:
                    body()

    nc.compile()
    return nc


# ----------------------------------------------------------------- interface

_CACHE = {}

def _get_nc(static):
    key = (static["N"], static["D"], static["n_cores"], static["CHT"],
           static["n_ag"], tuple(static["K"].reshape(-1).tolist()))
    if key not in _CACHE:
        _CACHE[key] = _build(static)
    return _CACHE[key]


def kernel(x, deg, row, col):
    from concourse.bass_utils import run_bass_kernel_spmd
    x = np.asarray(x, dtype=np.float32)
    deg = np.asarray(deg, dtype=np.float32)
    row = np.asarray(row).astype(np.int64)
    col = np.asarray(col).astype(np.int64)
    N, D = x.shape
    n_cores = FULL_CORES
    static, ins = _plan(N, D, n_cores, row, col, deg, x)
    nc = _get_nc(static)
    in_maps = [{"x3pad": m["x3pad"], "x3o": m["x3o"], "idx16": m["idx16"],
                "rowmod": m["rowmod"], "drw": m["drw"], "gbc": m["gbc"]}
               for m in ins]
    try:
        res = run_bass_kernel_spmd(nc, in_maps, core_ids=list(range(n_cores)))
    except Exception:
        res = run_bass_kernel_spmd(nc, in_maps, core_ids=list(range(n_cores)))

    # host assembly: x3 exact fp32, y1/y2 from device bf16
    deg64 = deg.reshape(-1).astype(np.float64)
    sr = (1.0 / np.sqrt(deg64)).astype(np.float32)
    sq = np.sqrt(deg64).astype(np.float32)
    x3f = np.concatenate([x, x * sr[:, None], x * sq[:, None]], axis=1)
    out = np.empty((N, 3 * 3 * D), dtype=np.float32)
    out[:, 0:3 * D] = x3f
    npc = N // n_cores
    for c in range(n_cores):
        sl = slice(c * npc, (c + 1) * npc)
        out[sl, 3 * D:6 * D] = res.results[c]["y1o"][:, :3 * D].astype(np.float32)
        out[sl, 6 * D:9 * D] = res.results[c]["y2o"].astype(np.float32)
    return out.reshape(N, 9, D).transpose(0, 2, 1).copy()
